# revision 15
# baseline (speedup 1.0000x reference)
"""Trainium2 Bass kernel for nn_ConditionedS4DTransition.

Strategy (data-parallel over batch, 8 cores x 256 rows):
  - Whole pipeline runs feature-major ([features on partitions, batch on free dim])
    so BN scale/bias fold into host-preprocessed weights + per-partition ACT bias,
    and all matmuls are weight-stationary (lhsT = W k-tile, rhs = activationT).
  - Scan phase (serial over T=64): selector MLP -> Lam via tanh/sin identities
    (sigmoid(-a) = 0.5 - 0.5*tanh(a/2); cos(x) = sin(x+pi/2)) -> B-head matmuls
    -> per-(c,u)-tile products multiplied by udt (DVE bf16 2x) -> u-reduction on
    the TensorEngine via block-ones stationary matmuls accumulating into PSUM
    (partition-disjoint rows, bias matmul opens the accumulation group).
  - C/D heads (58% of FLOPs, not on the recurrence path) are deferred: hz/zn are
    spilled to HBM scratch in bf16 during the scan and re-processed in groups of
    4 steps with N=512 moving operands; same multiply+ones-matmul reduction
    produces Y feature-major, transposed back via the PE.
  - All matmul operands bf16 (fp32 accumulation in PSUM); recurrent state zn kept
    fp32 in SBUF. Measured end-to-end rel err vs fp32 reference: ~4e-3.
"""

import sys

sys.path.insert(0, "/opt/trn_rl_repo")

import numpy as np
import ml_dtypes

import concourse.bass as bass
import concourse.mybir as mybir
import concourse.tile as tile
from concourse import bacc
from concourse.bass_utils import run_bass_kernel_spmd

BF = ml_dtypes.bfloat16
F32 = mybir.dt.float32
BF16 = mybir.dt.bfloat16
AF = mybir.ActivationFunctionType
ALU = mybir.AluOpType

B, T_FULL = 2048, 64
DYN, CD, STAT, UD, NOBS, H = 256, 128, 128, 64, 80, 512
HZ = DYN + STAT  # 384
SEL_IN = DYN + STAT + UD + 1  # 449
EPS = 1e-5
NCORES = 8
BC = B // NCORES  # 256 batch rows per core
PI = float(np.pi)


def _pad_ktiles(w, k_logical, n):
    """[k_logical, n] -> [ceil(k/128), 128, n] zero-padded."""
    kt = (k_logical + 127) // 128
    out = np.zeros((kt * 128, n), np.float32)
    out[:k_logical] = w
    return out.reshape(kt, 128, n)


def build_kernel(T, dbg=False):
    nc = bacc.Bacc("TRN2", target_bir_lowering=False, debug=False)

    # ---- I/O declarations (per core) ----
    din = {}

    def inp(name, shape, dt):
        din[name] = nc.dram_tensor(name, list(shape), dt, kind="ExternalInput")
        return din[name]

    zR0T = inp("zR0T", [CD, BC], F32)
    zI0T = inp("zI0T", [CD, BC], F32)
    zR0Tb = inp("zR0Tb", [CD, BC], BF16)
    zI0Tb = inp("zI0Tb", [CD, BC], BF16)
    zstatTb = inp("zstatTb", [STAT, BC], BF16)
    sel4 = inp("sel4", [T, UD + 1, BC], BF16)  # [utT ; dtT] per step
    udtd = inp("udtd", [T, 128, BC], BF16)  # udt doubled (rows u, u again)

    W0s = inp("W0s", [4, 128, H], BF16)
    b0s = inp("b0s", [128, 4], F32)
    W1s = inp("W1s", [4, 128, H], BF16)
    b1s = inp("b1s", [128, 4], F32)
    W2s = inp("W2s", [4, 128, HZ], BF16)
    b2s = inp("b2s", [128, 3], F32)
    WaWos = inp("WaWos", [3, 128, 2 * CD], BF16)
    baos = inp("baos", [1, 2 * CD], BF16)
    WBrs = inp("WBrs", [3, 128, CD * UD], BF16)
    WBis = inp("WBis", [3, 128, CD * UD], BF16)
    bBrTs = inp("bBrTs", [UD, CD], BF16)
    bBiTs = inp("bBiTs", [UD, CD], BF16)
    WCrs = inp("WCrs", [3, 128, NOBS * CD], BF16)
    WCis = inp("WCis", [3, 128, NOBS * CD], BF16)
    WDs = inp("WDs", [3, 128, NOBS * UD], BF16)
    bCrTs = inp("bCrTs", [CD, NOBS], BF16)
    bCiTns = inp("bCiTns", [CD, NOBS], BF16)
    bDTs = inp("bDTs", [UD, NOBS], BF16)
    blkm = inp("blkm", [16, 128, 32], BF16)   # B/D-head reduce masks
    colm = inp("colm", [32, 128, 32], BF16)   # C-head reduce masks
    onesrow = inp("onesrow", [1, BC], BF16)
    identf = inp("identf", [128, 128], F32)
    identb = inp("identb", [128, 128], BF16)

    Zo = nc.dram_tensor("Z", [T, BC, DYN], F32, kind="ExternalOutput")
    Yo = nc.dram_tensor("Y", [T, BC, NOBS], F32, kind="ExternalOutput")
    DBGo = nc.dram_tensor("DBG", [12, 128, BC], F32, kind="ExternalOutput") \
        if dbg else None

    NB_B = (CD * UD) // 128  # 64 f-tiles per B-head
    NF_C = NOBS  # f-tiles per C-head (each = one o, all 128 c)
    NF_D = NOBS // 2  # f-tiles for D-head (two o per tile)

    from contextlib import ExitStack
    with tile.TileContext(nc) as tc, ExitStack() as stack:
        # ------------- persistent pools (both phases) -------------
        const = stack.enter_context(tc.tile_pool(name="const", bufs=1))
        dram = stack.enter_context(tc.tile_pool(name="dram", bufs=1, space="DRAM"))

        hz_scr = dram.tile([T, 3, 128, BC], BF16)
        zn_scr = dram.tile([T, 2, 128, BC], BF16)

        c_blkm = [const.tile([128, 32], BF16, tag=f"blkm{m}", name=f"blkm_{m}")
                  for m in range(16)]
        c_colm = [const.tile([128, 32], BF16, tag=f"colm{m}", name=f"colm_{m}")
                  for m in range(32)]
        c_onesrow = const.tile([1, BC], BF16)
        c_ident = const.tile([128, 128], F32)
        c_identb = const.tile([128, 128], BF16)
        c_zstat = const.tile([STAT, BC], BF16)
        c_pihalf = const.tile([128, 1], F32)
        nc.vector.memset(c_pihalf[:], PI / 2)
        for m in range(16):
            nc.sync.dma_start(out=c_blkm[m][:], in_=blkm[m])
        for m in range(32):
            nc.sync.dma_start(out=c_colm[m][:], in_=colm[m])
        nc.sync.dma_start(out=c_onesrow[:], in_=onesrow[:])
        nc.sync.dma_start(out=c_ident[:], in_=identf[:])
        nc.sync.dma_start(out=c_identb[:], in_=identb[:])
        nc.sync.dma_start(out=c_zstat[:], in_=zstatTb[:])

        # ======================= SCAN PHASE =======================
        with tc.tile_pool(name="swt", bufs=1) as swt, \
             tc.tile_pool(name="sst", bufs=1) as sst, \
             tc.tile_pool(name="sact", bufs=3) as sact, \
             tc.tile_pool(name="shz", bufs=2) as shz, \
             tc.tile_pool(name="ssmall", bufs=2) as ssmall, \
             tc.tile_pool(name="sprod", bufs=3) as sprod, \
             tc.tile_pool(name="szn", bufs=2) as sznp, \
             tc.tile_pool(name="sznb", bufs=2) as sznbp, \
             tc.tile_pool(name="szout", bufs=2) as szout, \
             tc.tile_pool(name="sio", bufs=2) as sio, \
             tc.tile_pool(name="pssel", bufs=2, space="PSUM") as pssel, \
             tc.tile_pool(name="pslam", bufs=1, space="PSUM") as pslam, \
             tc.tile_pool(name="psprod", bufs=2, space="PSUM") as psprod, \
             tc.tile_pool(name="psdrive", bufs=1, space="PSUM") as psdrive:

            # scan weights resident
            w0 = [swt.tile([128, H], BF16, tag=f"w0{k}", name=f"w0_{k}") for k in range(4)]
            w1 = [swt.tile([128, H], BF16, tag=f"w1{k}", name=f"w1_{k}") for k in range(4)]
            w2 = [swt.tile([128, HZ], BF16, tag=f"w2{k}", name=f"w2_{k}") for k in range(4)]
            wao = [swt.tile([128, 2 * CD], BF16, tag=f"wao{k}", name=f"wao_{k}") for k in range(3)]
            wbr = [swt.tile([128, CD * UD], BF16, tag=f"wbr{k}", name=f"wbr_{k}") for k in range(3)]
            wbi = [swt.tile([128, CD * UD], BF16, tag=f"wbi{k}", name=f"wbi_{k}") for k in range(3)]
            for k in range(4):
                nc.sync.dma_start(out=w0[k][:], in_=W0s[k])
                nc.sync.dma_start(out=w1[k][:], in_=W1s[k])
                nc.sync.dma_start(out=w2[k][:], in_=W2s[k])
            for k in range(3):
                nc.sync.dma_start(out=wao[k][:], in_=WaWos[k])
                nc.sync.dma_start(out=wbr[k][:], in_=WBrs[k])
                nc.sync.dma_start(out=wbi[k][:], in_=WBis[k])
            sb0 = sst.tile([128, 4], F32, tag="b0")
            sb1 = sst.tile([128, 4], F32, tag="b1")
            sb2 = sst.tile([128, 3], F32, tag="b2")
            sbao = sst.tile([1, 2 * CD], BF16, tag="bao")
            sbbr = sst.tile([UD, CD], BF16, tag="bbr")
            sbbi = sst.tile([UD, CD], BF16, tag="bbi")
            nc.sync.dma_start(out=sb0[:], in_=b0s[:])
            nc.sync.dma_start(out=sb1[:], in_=b1s[:])
            nc.sync.dma_start(out=sb2[:], in_=b2s[:])
            nc.sync.dma_start(out=sbao[:], in_=baos[:])
            nc.sync.dma_start(out=sbbr[:], in_=bBrTs[:])
            nc.sync.dma_start(out=sbbi[:], in_=bBiTs[:])

            # state (fp32 master, ping-pong) + bf16 cast tiles
            znR = sznp.tile([CD, BC], F32, tag="znR")
            znI = sznp.tile([CD, BC], F32, tag="znI")
            znRb = sznbp.tile([CD, BC], BF16, tag="znRb")
            znIb = sznbp.tile([CD, BC], BF16, tag="znIb")
            nc.sync.dma_start(out=znR[:], in_=zR0T[:])
            nc.sync.dma_start(out=znI[:], in_=zI0T[:])
            nc.sync.dma_start(out=znRb[:], in_=zR0Tb[:])
            nc.sync.dma_start(out=znIb[:], in_=zI0Tb[:])

            for t in range(T):
                # --- per-step inputs ---
                s4 = sio.tile([UD + 1, BC], BF16, tag="sel4")
                nc.sync.dma_start(out=s4[:], in_=sel4[t])
                ud2 = sio.tile([128, BC], BF16, tag="udtd")
                nc.sync.dma_start(out=ud2[:], in_=udtd[t])

                # --- selector MLP (feature-major) ---
                mov0 = [znRb, znIb, c_zstat, s4]
                h1 = []
                for f in range(4):
                    ps = pssel.tile([128, BC], F32, tag="sel")
                    for k in range(4):
                        kk = 128 if k < 3 else (UD + 1)
                        nc.tensor.matmul(
                            ps[:], w0[k][0:kk, f * 128:(f + 1) * 128],
                            mov0[k][0:kk, :], start=(k == 0), stop=(k == 3))
                    hb = sact.tile([128, BC], BF16, tag=f"h1{f}", name=f"h1_{f}")
                    nc.scalar.activation(hb[:], ps[:], AF.Relu, bias=sb0[:, f:f + 1])
                    h1.append(hb)
                h2 = []
                for f in range(4):
                    ps = pssel.tile([128, BC], F32, tag="sel")
                    for k in range(4):
                        nc.tensor.matmul(
                            ps[:], w1[k][:, f * 128:(f + 1) * 128],
                            h1[k][:], start=(k == 0), stop=(k == 3))
                    hb = sact.tile([128, BC], BF16, tag=f"h2{f}", name=f"h2_{f}")
                    nc.scalar.activation(hb[:], ps[:], AF.Relu, bias=sb1[:, f:f + 1])
                    h2.append(hb)
                hz = []
                for f in range(3):
                    ps = pssel.tile([128, BC], F32, tag="sel")
                    for k in range(4):
                        nc.tensor.matmul(
                            ps[:], w2[k][:, f * 128:(f + 1) * 128],
                            h2[k][:], start=(k == 0), stop=(k == 3))
                    hb = shz.tile([128, BC], BF16, tag=f"hz{f}", name=f"hz_{f}")
                    nc.scalar.activation(hb[:], ps[:], AF.Identity, bias=sb2[:, f:f + 1])
                    hz.append(hb)
                    nc.sync.dma_start(out=hz_scr[t, f], in_=hb[:])

                # --- Wa / Wo heads -> Lam ---
                alps = pslam.tile([128, BC], F32, tag="al")
                omps = pslam.tile([128, BC], F32, tag="om")
                for k in range(3):
                    nc.tensor.matmul(alps[:], wao[k][:, 0:CD], hz[k][:],
                                     start=(k == 0), stop=False)
                nc.tensor.matmul(alps[:], sbao[0:1, 0:CD], c_onesrow[:],
                                 start=False, stop=True)
                for k in range(3):
                    nc.tensor.matmul(omps[:], wao[k][:, CD:2 * CD], hz[k][:],
                                     start=(k == 0), stop=False)
                nc.tensor.matmul(omps[:], sbao[0:1, CD:2 * CD], c_onesrow[:],
                                 start=False, stop=True)
                th = ssmall.tile([128, BC], F32, tag="th")
                nc.scalar.activation(th[:], alps[:], AF.Tanh, scale=0.5)
                sinO = ssmall.tile([128, BC], F32, tag="sin")
                nc.scalar.activation(sinO[:], omps[:], AF.Sin)
                cosO = ssmall.tile([128, BC], F32, tag="cos")
                nc.scalar.activation(cosO[:], omps[:], AF.Sin, bias=c_pihalf[:])
                emag = ssmall.tile([128, BC], F32, tag="emag")
                nc.vector.tensor_scalar(emag[:], th[:], -0.5, 0.5, ALU.mult, ALU.add)
                lamR = ssmall.tile([128, BC], F32, tag="lamR")
                nc.vector.tensor_mul(lamR[:], emag[:], cosO[:])
                lamI = ssmall.tile([128, BC], F32, tag="lamI")
                nc.vector.tensor_mul(lamI[:], emag[:], sinO[:])

                # --- B head: drive = sum_u (hz@WB + bB)[c,u] * udt[u] ---
                zdR = psdrive.tile([128, BC], F32, tag="zdR")
                zdI = psdrive.tile([128, BC], F32, tag="zdI")
                nc.tensor.matmul(zdR[:], sbbr[:], ud2[0:UD, :], start=True,
                                 stop=False, skip_group_check=True)
                nc.tensor.matmul(zdI[:], sbbi[:], ud2[0:UD, :], start=True,
                                 stop=False, skip_group_check=True)
                for head, (wb, zd) in enumerate([(wbr, zdR), (wbi, zdI)]):
                    for ft in range(NB_B):
                        ps = psprod.tile([128, BC], F32, tag="prod")
                        for k in range(3):
                            nc.tensor.matmul(
                                ps[:], wb[k][:, ft * 128:(ft + 1) * 128],
                                hz[k][:], start=(k == 0), stop=(k == 2))
                        pb = sprod.tile([128, BC], BF16, tag="prb")
                        nc.scalar.activation(pb[:], ps[:], AF.Copy)
                        pm = sprod.tile([128, BC], BF16, tag="prm")
                        nc.vector.tensor_mul(pm[:], pb[:], ud2[:])
                        off = 32 * (ft // 16)
                        nc.tensor.matmul(
                            zd[off:off + 32, :], c_blkm[ft % 16][:], pm[:],
                            start=False, stop=(ft == NB_B - 1),
                            tile_position=(0, off), skip_group_check=True)

                # --- zn update (fp32) ---
                znR_n = sznp.tile([CD, BC], F32, tag="znR")
                znI_n = sznp.tile([CD, BC], F32, tag="znI")
                m1 = sprod.tile([CD, BC], F32, tag="m1")
                m2 = sprod.tile([CD, BC], F32, tag="m2")
                nc.vector.tensor_mul(m1[:], lamR[:], znR[:])
                nc.vector.tensor_mul(m2[:], lamI[:], znI[:])
                nc.vector.tensor_sub(m1[:], m1[:], m2[:])
                nc.vector.tensor_add(znR_n[:], m1[:], zdR[:])
                m3 = sprod.tile([CD, BC], F32, tag="m3")
                m4 = sprod.tile([CD, BC], F32, tag="m4")
                nc.vector.tensor_mul(m3[:], lamR[:], znI[:])
                nc.vector.tensor_mul(m4[:], lamI[:], znR[:])
                nc.vector.tensor_add(m3[:], m3[:], m4[:])
                nc.vector.tensor_add(znI_n[:], m3[:], zdI[:])

                znRb_n = sznbp.tile([CD, BC], BF16, tag="znRb")
                znIb_n = sznbp.tile([CD, BC], BF16, tag="znIb")
                nc.scalar.activation(znRb_n[:], znR_n[:], AF.Copy)
                nc.scalar.activation(znIb_n[:], znI_n[:], AF.Copy)
                nc.sync.dma_start(out=zn_scr[t, 0], in_=znRb_n[:])
                nc.sync.dma_start(out=zn_scr[t, 1], in_=znIb_n[:])

                # --- Z output: transpose to batch-major, DMA out ---
                for half in range(2):
                    ps = pssel.tile([128, 2 * CD], BF16, tag="sel", name="ztr")
                    cs = slice(half * 128, half * 128 + 128)
                    nc.tensor.transpose(ps[:, 0:CD], znRb_n[:, cs], c_identb[:])
                    nc.tensor.transpose(ps[:, CD:2 * CD], znIb_n[:, cs], c_identb[:])
                    zo = szout.tile([128, 2 * CD], F32, tag="zout")
                    nc.scalar.activation(zo[:], ps[:], AF.Copy)
                    nc.sync.dma_start(out=Zo[t, half * 128:half * 128 + 128, :],
                                      in_=zo[:])

                if dbg and t == 0:
                    def dcap(slot, ap):
                        dtile = szout.tile([128, BC], F32, tag="dbg",
                                           name=f"dbg{slot}")
                        nc.scalar.activation(dtile[:ap.shape[0]], ap, AF.Copy)
                        nc.sync.dma_start(out=DBGo[slot, 0:ap.shape[0], :],
                                          in_=dtile[:ap.shape[0]])
                    dcap(0, h1[0][:])
                    dcap(1, h2[0][:])
                    dcap(2, hz[0][:])
                    dcap(3, th[:])
                    dcap(4, sinO[:])
                    dcap(5, cosO[:])
                    dcap(6, emag[:])
                    dcap(7, lamR[:])
                    dcap(8, zdR[:])
                    dcap(9, zdI[:])
                    dcap(10, znR_n[:])
                    dcap(11, znI_n[:])
                znR, znI, znRb, znIb = znR_n, znI_n, znRb_n, znIb_n

        # ======================= PHASE 2: C/D heads =======================
        NG = T // 4
        with tc.tile_pool(name="pwt", bufs=1) as pwt, \
             tc.tile_pool(name="pio", bufs=2) as pio, \
             tc.tile_pool(name="pprod", bufs=3) as pprod, \
             tc.tile_pool(name="pyev", bufs=2) as pyev, \
             tc.tile_pool(name="psp2", bufs=3, space="PSUM") as psp2, \
             tc.tile_pool(name="psyt", bufs=2, space="PSUM") as psyt, \
             tc.tile_pool(name="psytr", bufs=2, space="PSUM") as psytr:

            wcr = [pwt.tile([128, NOBS * CD], BF16, tag=f"wcr{k}", name=f"wcr_{k}") for k in range(3)]
            wci = [pwt.tile([128, NOBS * CD], BF16, tag=f"wci{k}", name=f"wci_{k}") for k in range(3)]
            wd = [pwt.tile([128, NOBS * UD], BF16, tag=f"wd{k}", name=f"wd_{k}") for k in range(3)]
            for k in range(3):
                nc.sync.dma_start(out=wcr[k][:], in_=WCrs[k])
                nc.sync.dma_start(out=wci[k][:], in_=WCis[k])
                nc.sync.dma_start(out=wd[k][:], in_=WDs[k])
            pbcr = pwt.tile([CD, NOBS], BF16, tag="bcr")
            pbcin = pwt.tile([CD, NOBS], BF16, tag="bcin")
            pbd = pwt.tile([UD, NOBS], BF16, tag="bd")
            nc.sync.dma_start(out=pbcr[:], in_=bCrTs[:])
            nc.sync.dma_start(out=pbcin[:], in_=bCiTns[:])
            nc.sync.dma_start(out=pbd[:], in_=bDTs[:])

            for g in range(NG):
                ts0 = 4 * g
                hzk = [pio.tile([128, 4 * BC], BF16, tag=f"hzk{k}", name=f"hzk_{k}") for k in range(3)]
                for k in range(3):
                    for j in range(4):
                        nc.sync.dma_start(out=hzk[k][:, j * BC:(j + 1) * BC],
                                          in_=hz_scr[ts0 + j, k])
                znR2 = pio.tile([128, 4 * BC], BF16, tag="znR2")
                znI2 = pio.tile([128, 4 * BC], BF16, tag="znI2")
                udt2 = pio.tile([128, 4 * BC], BF16, tag="udt2")
                for j in range(4):
                    sl = slice(j * BC, (j + 1) * BC)
                    nc.sync.dma_start(out=znR2[:, sl], in_=zn_scr[ts0 + j, 0])
                    nc.sync.dma_start(out=znI2[:, sl], in_=zn_scr[ts0 + j, 1])
                    nc.sync.dma_start(out=udt2[:, sl], in_=udtd[ts0 + j])
                znI2n = pio.tile([128, 4 * BC], BF16, tag="znI2n")
                nc.vector.tensor_scalar_mul(znI2n[:], znI2[:], -1.0)

                yt = [psyt.tile([96, 512], F32, tag="yt", name=f"yt_{p}") for p in range(2)]
                for p in range(2):
                    sl = slice(p * 512, (p + 1) * 512)
                    nc.tensor.matmul(yt[p][0:NOBS, :], pbcr[:], znR2[:, sl],
                                     start=True, stop=False, skip_group_check=True)
                    nc.tensor.matmul(yt[p][0:NOBS, :], pbcin[:], znI2[:, sl],
                                     start=False, stop=False, skip_group_check=True)
                    nc.tensor.matmul(yt[p][0:NOBS, :], pbd[:], udt2[0:UD, sl],
                                     start=False, stop=False, skip_group_check=True)

                heads = [(wcr, znR2, NF_C, False), (wci, znI2n, NF_C, False),
                         (wd, udt2, NF_D, True)]
                for hi, (ws, mul_in, nf, is_d) in enumerate(heads):
                    last_head = hi == len(heads) - 1
                    for ft in range(nf):
                        for p in range(2):
                            sl = slice(p * 512, (p + 1) * 512)
                            ps = psp2.tile([128, 512], F32, tag="pr2")
                            for k in range(3):
                                nc.tensor.matmul(
                                    ps[:], ws[k][:, ft * 128:(ft + 1) * 128],
                                    hzk[k][:, sl], start=(k == 0), stop=(k == 2))
                            pb = pprod.tile([128, 512], BF16, tag="prb2")
                            nc.scalar.activation(pb[:], ps[:], AF.Copy)
                            pm = pprod.tile([128, 512], BF16, tag="prm2")
                            nc.vector.tensor_mul(pm[:], pb[:], mul_in[:, sl])
                            last = last_head and ft == nf - 1
                            if is_d:
                                off = 32 * (ft // 16)
                                nc.tensor.matmul(
                                    yt[p][off:off + 32, :], c_blkm[ft % 16][:],
                                    pm[:], start=False, stop=last,
                                    tile_position=(0, off), skip_group_check=True)
                            else:
                                off = 32 * (ft // 32)
                                nc.tensor.matmul(
                                    yt[p][off:off + 32, :], c_colm[ft % 32][:],
                                    pm[:], start=False, stop=last,
                                    tile_position=(0, off), skip_group_check=True)

                # evacuate Y: psum [80, 512] -> sbuf -> transpose -> [512, 80]
                for p in range(2):
                    ysb = pyev.tile([NOBS, 512], BF16, tag="ysb")
                    nc.scalar.activation(ysb[:], yt[p][0:NOBS, :], AF.Copy)
                    for j in range(4):
                        tp = psytr.tile([128, NOBS], BF16, tag="ytr")
                        nc.tensor.transpose(
                            tp[:], ysb[:, j * 128:(j + 1) * 128],
                            c_identb[0:NOBS, 0:NOBS])
                        yb = pyev.tile([128, NOBS], F32, tag="ybm")
                        nc.scalar.activation(yb[:], tp[:], AF.Copy)
                        nc.sync.dma_start(
                            out=Yo[ts0 + 2 * p + j // 2,
                                   (j % 2) * 128:(j % 2) * 128 + 128, :],
                            in_=yb[:])

    nc.compile()
    return nc


def _host_prep(T, inputs):
    """Preprocess weights (shared) and per-core inputs."""
    inv = np.float32(1.0 / np.sqrt(1.0 + EPS))
    f32 = lambda x: np.asarray(x, np.float32)
    W0 = f32(inputs["W0"]) * (inv * f32(inputs["g0"]))[None, :]
    b0 = f32(inputs["b0"]) * inv * f32(inputs["g0"]) + f32(inputs["be0"])
    W1 = f32(inputs["W1"]) * (inv * f32(inputs["g1"]))[None, :]
    b1 = f32(inputs["b1"]) * inv * f32(inputs["g1"]) + f32(inputs["be1"])
    W2, b2 = f32(inputs["W2"]), f32(inputs["b2"])
    blkm = np.zeros((16, 128, 32), np.float32)
    for m in range(16):
        blkm[m, 0:64, 2 * m] = 1.0
        blkm[m, 64:128, 2 * m + 1] = 1.0
    colm = np.zeros((32, 128, 32), np.float32)
    for m in range(32):
        colm[m, :, m] = 1.0
    shared = {
        "W0s": _pad_ktiles(W0, SEL_IN, H).astype(BF),
        "b0s": b0.reshape(4, 128).T.copy(),
        "W1s": _pad_ktiles(W1, H, H).astype(BF),
        "b1s": b1.reshape(4, 128).T.copy(),
        "W2s": _pad_ktiles(W2, H, HZ).astype(BF),
        "b2s": b2.reshape(3, 128).T.copy(),
        "WaWos": np.concatenate([f32(inputs["Wa"]), f32(inputs["Wo"])], 1)
            .reshape(3, 128, 2 * CD).astype(BF),
        "baos": np.concatenate([f32(inputs["ba"]), f32(inputs["bo"])])
            .reshape(1, 2 * CD).astype(BF),
        "WBrs": f32(inputs["WBr"]).reshape(3, 128, CD * UD).astype(BF),
        "WBis": f32(inputs["WBi"]).reshape(3, 128, CD * UD).astype(BF),
        "bBrTs": f32(inputs["bBr"]).reshape(CD, UD).T.astype(BF).copy(),
        "bBiTs": f32(inputs["bBi"]).reshape(CD, UD).T.astype(BF).copy(),
        "WCrs": f32(inputs["WCr"]).reshape(3, 128, NOBS * CD).astype(BF),
        "WCis": f32(inputs["WCi"]).reshape(3, 128, NOBS * CD).astype(BF),
        "WDs": f32(inputs["WD"]).reshape(3, 128, NOBS * UD).astype(BF),
        "bCrTs": f32(inputs["bCr"]).reshape(NOBS, CD).T.astype(BF).copy(),
        "bCiTns": (-f32(inputs["bCi"]).reshape(NOBS, CD).T).astype(BF).copy(),
        "bDTs": f32(inputs["bD"]).reshape(NOBS, UD).T.astype(BF).copy(),
        "blkm": blkm.astype(BF),
        "colm": colm.astype(BF),
        "onesrow": np.ones((1, BC), np.float32).astype(BF),
        "identf": np.eye(128, dtype=np.float32),
        "identb": np.eye(128, dtype=np.float32).astype(BF),
    }
    z_dyn = f32(inputs["z_dyn"])
    z_static = f32(inputs["z_static"])
    dt = f32(inputs["dt"])
    U = f32(inputs["U"])
    in_maps = []
    for c in range(NCORES):
        bs = slice(c * BC, (c + 1) * BC)
        zd = z_dyn[bs]
        zRT = np.ascontiguousarray(zd[:, :CD].T)
        zIT = np.ascontiguousarray(zd[:, CD:].T)
        dtc = dt[bs]  # [BC, 1]
        utT = np.ascontiguousarray(U[:T, bs, :].transpose(0, 2, 1))  # [T, 64, BC]
        sel4 = np.concatenate(
            [utT, np.broadcast_to(dtc.T[None], (T, 1, BC))], axis=1)  # [T, 65, BC]
        udt = utT * dtc.T[None]  # [T, 64, BC]
        udtd = np.concatenate([udt, udt], axis=1)  # [T, 128, BC]
        in_maps.append(dict(
            shared,
            zR0T=zRT, zI0T=zIT,
            zR0Tb=zRT.astype(BF), zI0Tb=zIT.astype(BF),
            zstatTb=np.ascontiguousarray(z_static[bs].T).astype(BF),
            sel4=sel4.astype(BF),
            udtd=udtd.astype(BF),
        ))
    return in_maps


_NC_CACHE = {}


def _get_nc(T):
    if T not in _NC_CACHE:
        _NC_CACHE[T] = build_kernel(T)
    return _NC_CACHE[T]


def kernel(**inputs):
    U = np.asarray(inputs["U"])
    T = U.shape[0]
    nc = _get_nc(T)
    in_maps = _host_prep(T, inputs)
    res = run_bass_kernel_spmd(nc, in_maps, core_ids=list(range(NCORES)))
    Z = np.empty((T, B, DYN), np.float32)
    Y = np.empty((T, B, NOBS), np.float32)
    for c in range(NCORES):
        bs = slice(c * BC, (c + 1) * BC)
        Z[:, bs, :] = res.results[c]["Z"]
        Y[:, bs, :] = res.results[c]["Y"]
    return Z, Y


# revision 27
# speedup vs baseline: 1.3229x; 1.3229x over previous
"""Trainium2 Bass kernel for nn_ConditionedS4DTransition.

Strategy (data-parallel over batch, 8 cores x 256 rows):
  - Whole pipeline runs feature-major ([features on partitions, batch on free dim])
    so BN scale/bias fold into host-preprocessed weights + per-partition ACT bias,
    and all matmuls are weight-stationary (lhsT = W k-tile, rhs = activationT).
  - Scan phase (serial over T=64): selector MLP -> Lam via tanh/sin identities
    (sigmoid(-a) = 0.5 - 0.5*tanh(a/2); cos(x) = sin(x+pi/2)) -> B-head matmuls
    -> per-(c,u)-tile products multiplied by udt (DVE bf16 2x) -> u-reduction on
    the TensorEngine via block-ones stationary matmuls accumulating into PSUM
    (partition-disjoint rows, bias matmul opens the accumulation group).
  - C/D heads (58% of FLOPs, not on the recurrence path) are deferred: hz/zn are
    spilled to HBM scratch in bf16 during the scan and re-processed in groups of
    4 steps with N=512 moving operands; same multiply+ones-matmul reduction
    produces Y feature-major, transposed back via the PE.
  - All matmul operands bf16 (fp32 accumulation in PSUM); recurrent state zn kept
    fp32 in SBUF. Measured end-to-end rel err vs fp32 reference: ~4e-3.
"""

import sys

sys.path.insert(0, "/opt/trn_rl_repo")

import numpy as np
import ml_dtypes

import concourse.bass as bass
import concourse.mybir as mybir
import concourse.tile as tile
from concourse import bacc
from concourse.bass_utils import run_bass_kernel_spmd

BF = ml_dtypes.bfloat16
F32 = mybir.dt.float32
BF16 = mybir.dt.bfloat16
AF = mybir.ActivationFunctionType
ALU = mybir.AluOpType

B, T_FULL = 2048, 64
DYN, CD, STAT, UD, NOBS, H = 256, 128, 128, 64, 80, 512
HZ = DYN + STAT  # 384
SEL_IN = DYN + STAT + UD + 1  # 449
EPS = 1e-5
NCORES = 8
BC = B // NCORES  # 256 batch rows per core
PI = float(np.pi)


def _pad_ktiles(w, k_logical, n):
    """[k_logical, n] -> [ceil(k/128), 128, n] zero-padded."""
    kt = (k_logical + 127) // 128
    out = np.zeros((kt * 128, n), np.float32)
    out[:k_logical] = w
    return out.reshape(kt, 128, n)


def build_kernel(T, dbg=False, do_scan=True, do_p2=True):
    nc = bacc.Bacc("TRN2", target_bir_lowering=False, debug=False)

    # ---- I/O declarations (per core) ----
    din = {}

    def inp(name, shape, dt):
        din[name] = nc.dram_tensor(name, list(shape), dt, kind="ExternalInput")
        return din[name]

    zR0T = inp("zR0T", [CD, BC], F32)
    zI0T = inp("zI0T", [CD, BC], F32)
    zR0Tb = inp("zR0Tb", [CD, BC], BF16)
    zI0Tb = inp("zI0Tb", [CD, BC], BF16)
    zstatTb = inp("zstatTb", [STAT, BC], BF16)
    sel4 = inp("sel4", [T, UD + 1, BC], BF16)  # [utT ; dtT] per step
    udtd = inp("udtd", [T, 128, BC], BF16)  # udt doubled (rows u, u again)

    W0s = inp("W0s", [4, 128, H], BF16)
    b0s = inp("b0s", [128, 4], F32)
    W1s = inp("W1s", [4, 128, H], BF16)
    b1s = inp("b1s", [128, 4], F32)
    W2s = inp("W2s", [4, 128, HZ], BF16)
    b2s = inp("b2s", [128, 3], F32)
    WaWos = inp("WaWos", [3, 128, 2 * CD], BF16)
    baos = inp("baos", [1, 2 * CD], BF16)
    WBrs = inp("WBrs", [3, 128, CD * UD], BF16)
    WBis = inp("WBis", [3, 128, CD * UD], BF16)
    bBrTs = inp("bBrTs", [UD, CD], BF16)
    bBiTs = inp("bBiTs", [UD, CD], BF16)
    WCrs = inp("WCrs", [3, 128, NOBS * CD], BF16)
    WCis = inp("WCis", [3, 128, NOBS * CD], BF16)
    WDs = inp("WDs", [3, 128, NOBS * UD], BF16)
    bCrTs = inp("bCrTs", [CD, NOBS], BF16)
    bCiTns = inp("bCiTns", [CD, NOBS], BF16)
    bDTs = inp("bDTs", [UD, NOBS], BF16)
    blkm = inp("blkm", [16, 128, 32], BF16)   # B/D-head reduce masks
    colm = inp("colm", [32, 128, 32], BF16)   # C-head reduce masks
    onesrow = inp("onesrow", [1, BC], BF16)
    identf = inp("identf", [128, 128], F32)
    identb = inp("identb", [128, 128], BF16)

    Zo = nc.dram_tensor("Z", [T, BC, DYN], F32, kind="ExternalOutput")
    Yo = nc.dram_tensor("Y", [T, BC, NOBS], F32, kind="ExternalOutput")
    DBGo = nc.dram_tensor("DBG", [12, 128, BC], F32, kind="ExternalOutput") \
        if dbg else None

    NB_B = (CD * UD) // 128  # 64 f-tiles per B-head
    NF_C = NOBS  # f-tiles per C-head (each = one o, all 128 c)
    NF_D = NOBS // 2  # f-tiles for D-head (two o per tile)

    from contextlib import ExitStack
    with tile.TileContext(nc) as tc, ExitStack() as stack:
        # ------------- persistent pools (both phases) -------------
        const = stack.enter_context(tc.tile_pool(name="const", bufs=1))
        dram = stack.enter_context(tc.tile_pool(name="dram", bufs=1, space="DRAM"))

        hz_scr = dram.tile([T, 3, 128, BC], BF16)
        zn_scr = dram.tile([T, 2, 128, BC], BF16)

        c_blkm = [const.tile([128, 32], BF16, tag=f"blkm{m}", name=f"blkm_{m}")
                  for m in range(16)]
        c_colm = [const.tile([128, 32], BF16, tag=f"colm{m}", name=f"colm_{m}")
                  for m in range(32)]
        c_onesrow = const.tile([1, BC], BF16)
        c_ident = const.tile([128, 128], F32)
        c_identb = const.tile([128, 128], BF16)
        c_zstat = const.tile([STAT, BC], BF16)
        c_pihalf = const.tile([128, 1], F32)
        nc.vector.memset(c_pihalf[:], PI / 2)
        for m in range(16):
            nc.sync.dma_start(out=c_blkm[m][:], in_=blkm[m])
        for m in range(32):
            nc.sync.dma_start(out=c_colm[m][:], in_=colm[m])
        nc.sync.dma_start(out=c_onesrow[:], in_=onesrow[:])
        nc.sync.dma_start(out=c_ident[:], in_=identf[:])
        nc.sync.dma_start(out=c_identb[:], in_=identb[:])
        nc.sync.dma_start(out=c_zstat[:], in_=zstatTb[:])

        # ======================= SCAN PHASE =======================
        with tc.tile_pool(name="swt", bufs=1) as swt, \
             tc.tile_pool(name="sst", bufs=1) as sst, \
             tc.tile_pool(name="sact", bufs=3) as sact, \
             tc.tile_pool(name="shz", bufs=2) as shz, \
             tc.tile_pool(name="ssmall", bufs=2) as ssmall, \
             tc.tile_pool(name="sprod", bufs=6) as sprod, \
             tc.tile_pool(name="szn", bufs=2) as sznp, \
             tc.tile_pool(name="sznb", bufs=2) as sznbp, \
             tc.tile_pool(name="szout", bufs=2) as szout, \
             tc.tile_pool(name="sio", bufs=2) as sio, \
             tc.tile_pool(name="pssel", bufs=2, space="PSUM") as pssel, \
             tc.tile_pool(name="psprod", bufs=4, space="PSUM") as psprod, \
             tc.tile_pool(name="psdrive", bufs=1, space="PSUM") as psdrive:

            # scan weights resident
            w0 = [swt.tile([128, H], BF16, tag=f"w0{k}", name=f"w0_{k}") for k in range(4)]
            w1 = [swt.tile([128, H], BF16, tag=f"w1{k}", name=f"w1_{k}") for k in range(4)]
            w2 = [swt.tile([128, HZ], BF16, tag=f"w2{k}", name=f"w2_{k}") for k in range(4)]
            wao = [swt.tile([128, 2 * CD], BF16, tag=f"wao{k}", name=f"wao_{k}") for k in range(3)]
            wbr = [swt.tile([128, CD * UD], BF16, tag=f"wbr{k}", name=f"wbr_{k}") for k in range(3)]
            wbi = [swt.tile([128, CD * UD], BF16, tag=f"wbi{k}", name=f"wbi_{k}") for k in range(3)]
            for k in range(4):
                nc.sync.dma_start(out=w0[k][:], in_=W0s[k])
                nc.sync.dma_start(out=w1[k][:], in_=W1s[k])
                nc.sync.dma_start(out=w2[k][:], in_=W2s[k])
            for k in range(3):
                nc.sync.dma_start(out=wao[k][:], in_=WaWos[k])
                nc.sync.dma_start(out=wbr[k][:], in_=WBrs[k])
                nc.sync.dma_start(out=wbi[k][:], in_=WBis[k])
            sb0 = sst.tile([128, 4], F32, tag="b0")
            sb1 = sst.tile([128, 4], F32, tag="b1")
            sb2 = sst.tile([128, 3], F32, tag="b2")
            sbao = sst.tile([1, 2 * CD], BF16, tag="bao")
            sbbr = sst.tile([UD, CD], BF16, tag="bbr")
            sbbi = sst.tile([UD, CD], BF16, tag="bbi")
            nc.sync.dma_start(out=sb0[:], in_=b0s[:])
            nc.sync.dma_start(out=sb1[:], in_=b1s[:])
            nc.sync.dma_start(out=sb2[:], in_=b2s[:])
            nc.sync.dma_start(out=sbao[:], in_=baos[:])
            nc.sync.dma_start(out=sbbr[:], in_=bBrTs[:])
            nc.sync.dma_start(out=sbbi[:], in_=bBiTs[:])

            # state (fp32 master, ping-pong) + bf16 cast tiles
            znR = sznp.tile([CD, BC], F32, tag="znR")
            znI = sznp.tile([CD, BC], F32, tag="znI")
            znRb = sznbp.tile([CD, BC], BF16, tag="znRb")
            znIb = sznbp.tile([CD, BC], BF16, tag="znIb")
            nc.sync.dma_start(out=znR[:], in_=zR0T[:])
            nc.sync.dma_start(out=znI[:], in_=zI0T[:])
            nc.sync.dma_start(out=znRb[:], in_=zR0Tb[:])
            nc.sync.dma_start(out=znIb[:], in_=zI0Tb[:])

            for t in range(T if do_scan else 0):
                # --- per-step inputs ---
                s4 = sio.tile([UD + 1, BC], BF16, tag="sel4")
                nc.sync.dma_start(out=s4[:], in_=sel4[t])
                ud2 = sio.tile([128, BC], BF16, tag="udtd")
                nc.sync.dma_start(out=ud2[:], in_=udtd[t])

                # --- selector MLP (feature-major) ---
                mov0 = [znRb, znIb, c_zstat, s4]
                h1 = []
                korder = (2, 3, 0, 1)
                for f in range(4):
                    ps = pssel.tile([128, BC], F32, tag="sel")
                    for ki, k in enumerate(korder):
                        kk = 128 if k < 3 else (UD + 1)
                        nc.tensor.matmul(
                            ps[:], w0[k][0:kk, f * 128:(f + 1) * 128],
                            mov0[k][0:kk, :], start=(ki == 0), stop=(ki == 3))
                    hb = sact.tile([128, BC], BF16, tag=f"h1{f}", name=f"h1_{f}")
                    nc.scalar.activation(hb[:], ps[:], AF.Relu, bias=sb0[:, f:f + 1])
                    h1.append(hb)
                h2 = []
                for f in range(4):
                    ps = pssel.tile([128, BC], F32, tag="sel")
                    for k in range(4):
                        nc.tensor.matmul(
                            ps[:], w1[k][:, f * 128:(f + 1) * 128],
                            h1[k][:], start=(k == 0), stop=(k == 3))
                    hb = sact.tile([128, BC], BF16, tag=f"h2{f}", name=f"h2_{f}")
                    nc.scalar.activation(hb[:], ps[:], AF.Relu, bias=sb1[:, f:f + 1])
                    h2.append(hb)
                hz = []
                for f in range(3):
                    ps = pssel.tile([128, BC], F32, tag="sel")
                    for k in range(4):
                        nc.tensor.matmul(
                            ps[:], w2[k][:, f * 128:(f + 1) * 128],
                            h2[k][:], start=(k == 0), stop=(k == 3))
                    hb = shz.tile([128, BC], BF16, tag=f"hz{f}", name=f"hz_{f}")
                    nc.scalar.activation(hb[:], ps[:], AF.Identity, bias=sb2[:, f:f + 1])
                    hz.append(hb)
                    nc.sync.dma_start(out=hz_scr[t, f], in_=hb[:])

                # --- Wa / Wo heads -> Lam ---
                alom = pssel.tile([128, 2 * BC], F32, tag="sel", name="alom")
                alps = alom[:, 0:BC]
                omps = alom[:, BC:2 * BC]
                mm_a0 = nc.tensor.matmul(alps, wao[0][:, 0:CD], hz[0][:],
                                         start=True, stop=False,
                                         skip_group_check=True)
                for k in (1, 2):
                    nc.tensor.matmul(alps, wao[k][:, 0:CD], hz[k][:],
                                     start=False, stop=False,
                                     skip_group_check=True)
                nc.tensor.matmul(alps, sbao[0:1, 0:CD], c_onesrow[:],
                                 start=False, stop=False, skip_group_check=True)
                mm_o0 = nc.tensor.matmul(omps, wao[0][:, CD:2 * CD], hz[0][:],
                                         start=False, stop=False,
                                         skip_group_check=True)
                tile.add_dep_helper(mm_o0.ins, mm_a0.ins, sync=False,
                                    reason="alom has_written order")
                for k in (1, 2):
                    nc.tensor.matmul(omps, wao[k][:, CD:2 * CD], hz[k][:],
                                     start=False, stop=False,
                                     skip_group_check=True)
                nc.tensor.matmul(omps, sbao[0:1, CD:2 * CD], c_onesrow[:],
                                 start=False, stop=True, skip_group_check=True)
                th = ssmall.tile([128, BC], F32, tag="th")
                nc.scalar.activation(th[:], alps, AF.Tanh, scale=0.5)
                sinO = ssmall.tile([128, BC], F32, tag="sin")
                nc.scalar.activation(sinO[:], omps, AF.Sin)
                cosO = ssmall.tile([128, BC], F32, tag="cos")
                nc.scalar.activation(cosO[:], omps, AF.Sin, bias=c_pihalf[:])
                emag = ssmall.tile([128, BC], F32, tag="emag")
                nc.vector.tensor_scalar(emag[:], th[:], -0.5, 0.5, ALU.mult, ALU.add)
                lamR = ssmall.tile([128, BC], F32, tag="lamR")
                nc.vector.tensor_mul(lamR[:], emag[:], cosO[:])
                lamI = ssmall.tile([128, BC], F32, tag="lamI")
                nc.vector.tensor_mul(lamI[:], emag[:], sinO[:])

                # --- B head: drive = sum_u (hz@WB + bB)[c,u] * udt[u] ---
                zdrv = psdrive.tile([128, 2 * BC], F32, tag="drive")
                zdR = zdrv[:, 0:BC]
                zdI = zdrv[:, BC:2 * BC]
                mm_br = nc.tensor.matmul(zdR, sbbr[:], ud2[0:UD, :], start=True,
                                         stop=False, skip_group_check=True)
                mm_bi = nc.tensor.matmul(zdI, sbbi[:], ud2[0:UD, :], start=False,
                                         stop=False, skip_group_check=True)
                tile.add_dep_helper(mm_bi.ins, mm_br.ins, sync=False,
                                    reason="drive has_written order")
                pending = []

                def flush_pending():
                    for (pm_, co_, ft_, last_) in pending:
                        off = 32 * (ft_ // 16)
                        nc.tensor.matmul(
                            zdrv[off:off + 32, co_:co_ + BC],
                            c_blkm[ft_ % 16][:], pm_[:],
                            start=False, stop=last_,
                            tile_position=(0, off), skip_group_check=True)
                    pending.clear()

                for head, (wb, co) in enumerate([(wbr, 0), (wbi, BC)]):
                    for ft in range(NB_B):
                        ps = psprod.tile([128, BC], F32, tag="prod")
                        for k in range(3):
                            nc.tensor.matmul(
                                ps[:], wb[k][:, ft * 128:(ft + 1) * 128],
                                hz[k][:], start=(k == 0), stop=(k == 2))
                        pb = sprod.tile([128, BC], BF16, tag="prb")
                        nc.scalar.activation(pb[:], ps[:], AF.Copy)
                        pm = sprod.tile([128, BC], BF16, tag="prm")
                        nc.vector.tensor_mul(pm[:], pb[:], ud2[:])
                        pending.append(
                            (pm, co, ft, head == 1 and ft == NB_B - 1))
                        if len(pending) >= 3:
                            flush_pending()
                flush_pending()

                # --- zn update (fp32) ---
                znR_n = sznp.tile([CD, BC], F32, tag="znR")
                znI_n = sznp.tile([CD, BC], F32, tag="znI")
                m1 = sprod.tile([CD, BC], F32, tag="m1")
                m2 = sprod.tile([CD, BC], F32, tag="m2")
                nc.vector.tensor_mul(m1[:], lamR[:], znR[:])
                nc.vector.tensor_mul(m2[:], lamI[:], znI[:])
                nc.vector.tensor_sub(m1[:], m1[:], m2[:])
                nc.vector.tensor_add(znR_n[:], m1[:], zdR)
                m3 = sprod.tile([CD, BC], F32, tag="m3")
                m4 = sprod.tile([CD, BC], F32, tag="m4")
                nc.vector.tensor_mul(m3[:], lamR[:], znI[:])
                nc.vector.tensor_mul(m4[:], lamI[:], znR[:])
                nc.vector.tensor_add(m3[:], m3[:], m4[:])
                nc.vector.tensor_add(znI_n[:], m3[:], zdI)

                znRb_n = sznbp.tile([CD, BC], BF16, tag="znRb")
                znIb_n = sznbp.tile([CD, BC], BF16, tag="znIb")
                nc.scalar.activation(znRb_n[:], znR_n[:], AF.Copy)
                nc.scalar.activation(znIb_n[:], znI_n[:], AF.Copy)
                nc.sync.dma_start(out=zn_scr[t, 0], in_=znRb_n[:])
                nc.sync.dma_start(out=zn_scr[t, 1], in_=znIb_n[:])

                # --- Z output: transpose to batch-major, DMA out ---
                for half in range(2):
                    ps = pssel.tile([128, 2 * CD], BF16, tag="sel", name="ztr")
                    cs = slice(half * 128, half * 128 + 128)
                    nc.tensor.transpose(ps[:, 0:CD], znRb_n[:, cs], c_identb[:])
                    nc.tensor.transpose(ps[:, CD:2 * CD], znIb_n[:, cs], c_identb[:])
                    zo = szout.tile([128, 2 * CD], F32, tag="zout")
                    nc.scalar.activation(zo[:], ps[:], AF.Copy)
                    nc.sync.dma_start(out=Zo[t, half * 128:half * 128 + 128, :],
                                      in_=zo[:])

                if dbg and t == 0:
                    def dcap(slot, ap):
                        dtile = szout.tile([128, BC], F32, tag="dbg",
                                           name=f"dbg{slot}")
                        nc.scalar.activation(dtile[:ap.shape[0]], ap, AF.Copy)
                        nc.sync.dma_start(out=DBGo[slot, 0:ap.shape[0], :],
                                          in_=dtile[:ap.shape[0]])
                    dcap(0, h1[0][:])
                    dcap(1, h2[0][:])
                    dcap(2, hz[0][:])
                    dcap(3, th[:])
                    dcap(4, sinO[:])
                    dcap(5, cosO[:])
                    dcap(6, emag[:])
                    dcap(7, lamR[:])
                    dcap(8, zdR)
                    dcap(9, zdI)
                    dcap(10, znR_n[:])
                    dcap(11, znI_n[:])
                znR, znI, znRb, znIb = znR_n, znI_n, znRb_n, znIb_n

        # ======================= PHASE 2: C/D heads =======================
        NG = T // 4
        with tc.tile_pool(name="pwt", bufs=1) as pwt, \
             tc.tile_pool(name="pio", bufs=2) as pio, \
             tc.tile_pool(name="pprod", bufs=6) as pprod, \
             tc.tile_pool(name="pyev", bufs=2) as pyev, \
             tc.tile_pool(name="psp2", bufs=4, space="PSUM") as psp2, \
             tc.tile_pool(name="psyt", bufs=2, space="PSUM") as psyt, \
             tc.tile_pool(name="psytr", bufs=2, space="PSUM") as psytr:

            wcr = [pwt.tile([128, NOBS * CD], BF16, tag=f"wcr{k}", name=f"wcr_{k}") for k in range(3)]
            wci = [pwt.tile([128, NOBS * CD], BF16, tag=f"wci{k}", name=f"wci_{k}") for k in range(3)]
            wd = [pwt.tile([128, NOBS * UD], BF16, tag=f"wd{k}", name=f"wd_{k}") for k in range(3)]
            for k in range(3):
                nc.sync.dma_start(out=wcr[k][:], in_=WCrs[k])
                nc.sync.dma_start(out=wci[k][:], in_=WCis[k])
                nc.sync.dma_start(out=wd[k][:], in_=WDs[k])
            pbcr = pwt.tile([CD, NOBS], BF16, tag="bcr")
            pbcin = pwt.tile([CD, NOBS], BF16, tag="bcin")
            pbd = pwt.tile([UD, NOBS], BF16, tag="bd")
            nc.sync.dma_start(out=pbcr[:], in_=bCrTs[:])
            nc.sync.dma_start(out=pbcin[:], in_=bCiTns[:])
            nc.sync.dma_start(out=pbd[:], in_=bDTs[:])

            for g in range(NG if do_p2 else 0):
                ts0 = 4 * g
                hzk = [pio.tile([128, 4 * BC], BF16, tag=f"hzk{k}", name=f"hzk_{k}") for k in range(3)]
                for k in range(3):
                    for j in range(4):
                        nc.sync.dma_start(out=hzk[k][:, j * BC:(j + 1) * BC],
                                          in_=hz_scr[ts0 + j, k])
                znR2 = pio.tile([128, 4 * BC], BF16, tag="znR2")
                znI2 = pio.tile([128, 4 * BC], BF16, tag="znI2")
                udt2 = pio.tile([128, 4 * BC], BF16, tag="udt2")
                for j in range(4):
                    sl = slice(j * BC, (j + 1) * BC)
                    nc.sync.dma_start(out=znR2[:, sl], in_=zn_scr[ts0 + j, 0])
                    nc.sync.dma_start(out=znI2[:, sl], in_=zn_scr[ts0 + j, 1])
                    nc.sync.dma_start(out=udt2[:, sl], in_=udtd[ts0 + j])
                znI2n = pio.tile([128, 4 * BC], BF16, tag="znI2n")
                nc.vector.tensor_scalar_mul(znI2n[:], znI2[:], -1.0)

                yt = [psyt.tile([96, 512], F32, tag="yt", name=f"yt_{p}") for p in range(2)]
                for p in range(2):
                    sl = slice(p * 512, (p + 1) * 512)
                    nc.tensor.matmul(yt[p][0:NOBS, :], pbcr[:], znR2[:, sl],
                                     start=True, stop=False, skip_group_check=True)
                    nc.tensor.matmul(yt[p][0:NOBS, :], pbcin[:], znI2[:, sl],
                                     start=False, stop=False, skip_group_check=True)
                    nc.tensor.matmul(yt[p][0:NOBS, :], pbd[:], udt2[0:UD, sl],
                                     start=False, stop=False, skip_group_check=True)

                heads = [(wcr, znR2, NF_C, False), (wci, znI2n, NF_C, False),
                         (wd, udt2, NF_D, True)]
                pend2 = []

                def flush2():
                    for (pm_, p_, ft_, is_d_, last_) in pend2:
                        if is_d_:
                            off = 32 * (ft_ // 16)
                            msk = c_blkm[ft_ % 16]
                        else:
                            off = 32 * (ft_ // 32)
                            msk = c_colm[ft_ % 32]
                        nc.tensor.matmul(
                            yt[p_][off:off + 32, :], msk[:], pm_,
                            start=False, stop=last_,
                            tile_position=(0, off), skip_group_check=True)
                    pend2.clear()

                for hi, (ws, mul_in, nf, is_d) in enumerate(heads):
                    last_head = hi == len(heads) - 1
                    for ft in range(nf):
                        for p in range(2):
                            sl = slice(p * 512, (p + 1) * 512)
                            ps = psp2.tile([128, 512], F32, tag="pr2")
                            for k in range(3):
                                nc.tensor.matmul(
                                    ps[:], ws[k][:, ft * 128:(ft + 1) * 128],
                                    hzk[k][:, sl], start=(k == 0), stop=(k == 2))
                            pb = pprod.tile([128, 512], BF16, tag="prb2")
                            nc.scalar.activation(pb[:], ps[:], AF.Copy)
                            pm = pprod.tile([128, 512], BF16, tag="prm2")
                            nc.vector.tensor_mul(pm[:], pb[:], mul_in[:, sl])
                            pend2.append(
                                (pm[:], p, ft, is_d, last_head and ft == nf - 1))
                            if len(pend2) >= 6:
                                flush2()
                flush2()

                # evacuate Y: psum [80, 512] -> sbuf -> transpose -> [512, 80]
                for p in range(2):
                    ysb = pyev.tile([NOBS, 512], BF16, tag="ysb")
                    nc.scalar.activation(ysb[:], yt[p][0:NOBS, :], AF.Copy)
                    for j in range(4):
                        tp = psytr.tile([128, NOBS], BF16, tag="ytr")
                        nc.tensor.transpose(
                            tp[:], ysb[:, j * 128:(j + 1) * 128],
                            c_identb[0:NOBS, 0:NOBS])
                        yb = pyev.tile([128, NOBS], F32, tag="ybm")
                        nc.scalar.activation(yb[:], tp[:], AF.Copy)
                        nc.sync.dma_start(
                            out=Yo[ts0 + 2 * p + j // 2,
                                   (j % 2) * 128:(j % 2) * 128 + 128, :],
                            in_=yb[:])

    nc.compile()
    return nc


def _host_prep(T, inputs):
    """Preprocess weights (shared) and per-core inputs."""
    inv = np.float32(1.0 / np.sqrt(1.0 + EPS))
    f32 = lambda x: np.asarray(x, np.float32)
    W0 = f32(inputs["W0"]) * (inv * f32(inputs["g0"]))[None, :]
    b0 = f32(inputs["b0"]) * inv * f32(inputs["g0"]) + f32(inputs["be0"])
    W1 = f32(inputs["W1"]) * (inv * f32(inputs["g1"]))[None, :]
    b1 = f32(inputs["b1"]) * inv * f32(inputs["g1"]) + f32(inputs["be1"])
    W2, b2 = f32(inputs["W2"]), f32(inputs["b2"])
    blkm = np.zeros((16, 128, 32), np.float32)
    for m in range(16):
        blkm[m, 0:64, 2 * m] = 1.0
        blkm[m, 64:128, 2 * m + 1] = 1.0
    colm = np.zeros((32, 128, 32), np.float32)
    for m in range(32):
        colm[m, :, m] = 1.0
    shared = {
        "W0s": _pad_ktiles(W0, SEL_IN, H).astype(BF),
        "b0s": b0.reshape(4, 128).T.copy(),
        "W1s": _pad_ktiles(W1, H, H).astype(BF),
        "b1s": b1.reshape(4, 128).T.copy(),
        "W2s": _pad_ktiles(W2, H, HZ).astype(BF),
        "b2s": b2.reshape(3, 128).T.copy(),
        "WaWos": np.concatenate([f32(inputs["Wa"]), f32(inputs["Wo"])], 1)
            .reshape(3, 128, 2 * CD).astype(BF),
        "baos": np.concatenate([f32(inputs["ba"]), f32(inputs["bo"])])
            .reshape(1, 2 * CD).astype(BF),
        "WBrs": f32(inputs["WBr"]).reshape(3, 128, CD * UD).astype(BF),
        "WBis": f32(inputs["WBi"]).reshape(3, 128, CD * UD).astype(BF),
        "bBrTs": f32(inputs["bBr"]).reshape(CD, UD).T.astype(BF).copy(),
        "bBiTs": f32(inputs["bBi"]).reshape(CD, UD).T.astype(BF).copy(),
        "WCrs": f32(inputs["WCr"]).reshape(3, 128, NOBS * CD).astype(BF),
        "WCis": f32(inputs["WCi"]).reshape(3, 128, NOBS * CD).astype(BF),
        "WDs": f32(inputs["WD"]).reshape(3, 128, NOBS * UD).astype(BF),
        "bCrTs": f32(inputs["bCr"]).reshape(NOBS, CD).T.astype(BF).copy(),
        "bCiTns": (-f32(inputs["bCi"]).reshape(NOBS, CD).T).astype(BF).copy(),
        "bDTs": f32(inputs["bD"]).reshape(NOBS, UD).T.astype(BF).copy(),
        "blkm": blkm.astype(BF),
        "colm": colm.astype(BF),
        "onesrow": np.ones((1, BC), np.float32).astype(BF),
        "identf": np.eye(128, dtype=np.float32),
        "identb": np.eye(128, dtype=np.float32).astype(BF),
    }
    z_dyn = f32(inputs["z_dyn"])
    z_static = f32(inputs["z_static"])
    dt = f32(inputs["dt"])
    U = f32(inputs["U"])
    in_maps = []
    for c in range(NCORES):
        bs = slice(c * BC, (c + 1) * BC)
        zd = z_dyn[bs]
        zRT = np.ascontiguousarray(zd[:, :CD].T)
        zIT = np.ascontiguousarray(zd[:, CD:].T)
        dtc = dt[bs]  # [BC, 1]
        utT = np.ascontiguousarray(U[:T, bs, :].transpose(0, 2, 1))  # [T, 64, BC]
        sel4 = np.concatenate(
            [utT, np.broadcast_to(dtc.T[None], (T, 1, BC))], axis=1)  # [T, 65, BC]
        udt = utT * dtc.T[None]  # [T, 64, BC]
        udtd = np.concatenate([udt, udt], axis=1)  # [T, 128, BC]
        in_maps.append(dict(
            shared,
            zR0T=zRT, zI0T=zIT,
            zR0Tb=zRT.astype(BF), zI0Tb=zIT.astype(BF),
            zstatTb=np.ascontiguousarray(z_static[bs].T).astype(BF),
            sel4=sel4.astype(BF),
            udtd=udtd.astype(BF),
        ))
    return in_maps


_NC_CACHE = {}


def _get_nc(T):
    if T not in _NC_CACHE:
        _NC_CACHE[T] = build_kernel(T)
    return _NC_CACHE[T]


def kernel(**inputs):
    U = np.asarray(inputs["U"])
    T = U.shape[0]
    nc = _get_nc(T)
    in_maps = _host_prep(T, inputs)
    res = run_bass_kernel_spmd(nc, in_maps, core_ids=list(range(NCORES)))
    Z = np.empty((T, B, DYN), np.float32)
    Y = np.empty((T, B, NOBS), np.float32)
    for c in range(NCORES):
        bs = slice(c * BC, (c + 1) * BC)
        Z[:, bs, :] = res.results[c]["Z"]
        Y[:, bs, :] = res.results[c]["Y"]
    return Z, Y
